# revision 1
# baseline (speedup 1.0000x reference)
"""Trainium2 Bass kernel: Whisper-style self-attention (B=4, S=1500, D=1280, H=20).

Sharding: core c = 2*b + g handles batch b (of 4) and head-group g (of 2,
10 heads each).  Every matmul is exactly 1/8 of the total work:
  - Q/K/V projections column-sharded over the head group,
  - attention sharded by (batch, head),
  - output projection row-sharded; the two head-group partials of each batch
    are summed on the host (plus bias terms, which fold into host math).

Device dataflow (per core):
  xT [1280,1500] fp16 -> qT,kT [640,1500] f32r (qT scaled 1/8 + bq),
  v [1500,10,65] fp16 (64 v cols + ones col per head -> softmax Z),
  scoresT = kT.T@qT per (head, 500-col chunk) K=64, Exp batched over psum
  bank pairs on ACT -> expT fp16, ctxT accum in PSUM [65,500] over 12 sk
  tiles (row 64 = Z), DVE multiplies by gpsimd-broadcast 1/Z -> ctxT f32r,
  O-proj (f32r) -> out [1500,1280] f32.
Emission interleaves projections with attention units so ACT (exp) and PE
overlap; O-proj tiles are emitted as soon as their sq range is final.
"""
import sys
sys.path.insert(0, "/opt/trn_rl_repo")

from contextlib import ExitStack
import numpy as np

import concourse.bass as bass
import concourse.tile as tile
from concourse import bacc, mybir
from concourse.bass_utils import run_bass_kernel_spmd

dt = mybir.dt
AF = mybir.ActivationFunctionType
ALU = mybir.AluOpType

N_CORES = 8
B, S, D = 4, 1500, 1280
H, DH = 20, 64
G = 2
DG = D // G           # 640
HPG = H // G          # 10
KD = D // 128         # 10
MD = DG // 128        # 5
CW = (512, 512, 476)  # sq/proj chunk widths (PSUM-bank and f32r bound)
CO = (0, 512, 1024)   # chunk offsets
NS = 3
KS = (S + 127) // 128  # 12 (11*128 + 92)
ON = (512, 512, 256)

_CACHE = {}


def _sk(i):
    return min(128, S - i * 128)


def build():
    nc = bacc.Bacc("TRN2", target_bir_lowering=False, debug=False,
                   num_devices=N_CORES)
    xt_d = nc.dram_tensor("xt", [D, S], dt.float16, kind="ExternalInput").ap()
    wq_d = nc.dram_tensor("wq", [D, DG], dt.float16, kind="ExternalInput").ap()
    wk_d = nc.dram_tensor("wk", [D, DG], dt.float16, kind="ExternalInput").ap()
    wv_d = nc.dram_tensor("wv", [D, DG], dt.float16, kind="ExternalInput").ap()
    wo_d = nc.dram_tensor("wo", [DG, D], dt.float32, kind="ExternalInput").ap()
    bq_d = nc.dram_tensor("bq", [128, MD], dt.float32, kind="ExternalInput").ap()
    out_d = nc.dram_tensor("out", [S, D], dt.float32, kind="ExternalOutput").ap()

    xt_r = xt_d.rearrange("(k p) s -> p k s", p=128)
    wq_r = wq_d.rearrange("(k p) n -> p k n", p=128)
    wk_r = wk_d.rearrange("(k p) n -> p k n", p=128)
    wv_r = wv_d.rearrange("(k p) n -> p k n", p=128)
    wo_r = wo_d.rearrange("(k p) n -> p k n", p=128).bitcast(dt.float32r)

    with tile.TileContext(nc) as tc, ExitStack() as octx:
        persist = octx.enter_context(tc.tile_pool(name="persist", bufs=1))
        epool = octx.enter_context(tc.tile_pool(name="expT", bufs=3))
        zpool = octx.enter_context(tc.tile_pool(name="z", bufs=3))
        ps2 = octx.enter_context(tc.tile_pool(name="ps2", bufs=2, space="PSUM"))
        ps1 = octx.enter_context(tc.tile_pool(name="ps1", bufs=2, space="PSUM"))
        psc = octx.enter_context(tc.tile_pool(name="psc", bufs=2, space="PSUM"))

        qT = persist.tile([128, MD, S], dt.float32r, tag="qT")
        kT = persist.tile([128, MD, S], dt.float32r, tag="kT")
        v = persist.tile([128, KS, HPG, DH + 1], dt.float16, tag="v")
        ctxT = persist.tile([128, MD, S], dt.float32r, tag="ctxT")
        bq_s = persist.tile([128, MD], dt.float32, tag="bq")

        nc.sync.dma_start(out=bq_s[:], in_=bq_d[:])
        ones1 = persist.tile([128, 1], dt.float32, tag="ones1")
        nc.vector.memset(ones1[:], 1.0)
        nc.vector.tensor_copy(v[:, :, :, DH:DH + 1],
                              ones1[:].to_broadcast([128, KS, HPG, 1]))

        pb = ExitStack()
        xpool = pb.enter_context(tc.tile_pool(name="xt", bufs=1))
        wst = pb.enter_context(tc.tile_pool(name="wst", bufs=2))

        xt_s = xpool.tile([128, KD, S], dt.float16, tag="xt")
        for n in range(NS):
            nsl = slice(CO[n], CO[n] + CW[n])
            nc.sync.dma_start(out=xt_s[:, :, nsl], in_=xt_r[:, :, nsl])

        def emit_qk(m):
            """qT and kT for d-tile m (heads 2m, 2m+1)."""
            for w_r, dst, is_q in ((wq_r, qT, True), (wk_r, kT, False)):
                wt = wst.tile([128, KD, 320], dt.float16, tag="wst")
                nc.scalar.dma_start(out=wt[:, :, 0:128],
                                    in_=w_r[:, :, m * 128:(m + 1) * 128])
                for n in range(NS):
                    cw, co = CW[n], CO[n]
                    ps = ps1.tile([128, 1, 512], dt.float32, tag="ps1")
                    for kk in range(KD):
                        nc.tensor.matmul(
                            ps[:, 0, 0:cw],
                            lhsT=wt[:, kk, 0:128],
                            rhs=xt_s[:, kk, co:co + cw],
                            start=(kk == 0), stop=(kk == KD - 1))
                    osl = dst[:, m, co:co + cw]
                    if is_q:
                        nc.vector.tensor_scalar(
                            osl, ps[:, 0, 0:cw], 0.125, bq_s[:, m:m + 1],
                            op0=ALU.mult, op1=ALU.add)
                    else:
                        nc.vector.tensor_copy(osl, ps[:, 0, 0:cw])

        def emit_v(n):
            """v columns for heads 5n..5n+4 (+ their ones cols untouched)."""
            wt = wst.tile([128, KD, 320], dt.float16, tag="wst")
            nc.scalar.dma_start(out=wt[:], in_=wv_r[:, :, n * 320:(n + 1) * 320])
            for ms in range(KS):
                sp = _sk(ms)
                ps = ps1.tile([128, 1, 512], dt.float32, tag="ps1")
                for kk in range(KD):
                    nc.tensor.matmul(
                        ps[0:sp, 0, 0:320],
                        lhsT=xt_s[:, kk, ms * 128:ms * 128 + sp],
                        rhs=wt[:, kk, :],
                        start=(kk == 0), stop=(kk == KD - 1))
                nc.vector.tensor_copy(
                    v[0:sp, ms, n * 5:(n + 1) * 5, 0:DH],
                    ps[0:sp, 0, 0:320].rearrange("p (h e) -> p h e", h=5))

        def emit_unit(h, c):
            """Attention for head h, sq chunk c."""
            base = 64 * (h % 2)
            td = h // 2
            cw, co = CW[c], CO[c]
            csl = slice(co, co + cw)
            ex = epool.tile([128, KS, 512], dt.float16, tag="expT")
            for kk2 in range(0, KS, 2):
                ps = ps2.tile([128, 2, 512], dt.float32, tag="ps2")
                for j in range(2):
                    kk = kk2 + j
                    sp = _sk(kk)
                    nc.tensor.matmul(
                        ps[0:sp, j, 0:cw],
                        lhsT=kT[base:base + 64, td, kk * 128:kk * 128 + sp],
                        rhs=qT[base:base + 64, td, csl],
                        start=True, stop=True)
                nc.scalar.activation(ex[:, kk2:kk2 + 2, 0:cw], ps[:, :, 0:cw],
                                     AF.Exp)
            pc_t = psc.tile([DH + 1, 512], dt.float32, tag="psc")
            for kk in range(KS):
                sp = _sk(kk)
                nc.tensor.matmul(
                    pc_t[:, 0:cw],
                    lhsT=v[0:sp, kk, h, :],
                    rhs=ex[0:sp, kk, 0:cw],
                    start=(kk == 0), stop=(kk == KS - 1))
            zr = zpool.tile([1, 512], dt.float32, tag="zr")
            nc.vector.reciprocal(zr[:, 0:cw], pc_t[DH:DH + 1, 0:cw])
            zb = zpool.tile([64, 512], dt.float32, tag="zb")
            nc.gpsimd.partition_broadcast(zb[:, 0:cw], zr[:, 0:cw], channels=64)
            nc.vector.tensor_tensor(
                ctxT[base:base + 64, td, csl], pc_t[0:DH, 0:cw], zb[:, 0:cw],
                op=ALU.mult)

        emitted_oproj = [False] * KS

        def emit_oproj(ms_range, wo_s, opool):
            for ms in ms_range:
                if emitted_oproj[ms]:
                    continue
                emitted_oproj[ms] = True
                sp = _sk(ms)
                noff = 0
                for nw in ON:
                    ps = ps1.tile([128, 1, 512], dt.float32, tag="ps1")
                    for kk in range(MD):
                        nc.tensor.matmul(
                            ps[0:sp, 0, 0:nw],
                            lhsT=ctxT[:, kk, ms * 128:ms * 128 + sp],
                            rhs=wo_s[:, kk, noff:noff + nw],
                            start=(kk == 0), stop=(kk == MD - 1))
                    ob = opool.tile([128, 512], dt.float32, tag="ob")
                    nc.vector.tensor_copy(ob[0:sp, 0:nw], ps[0:sp, 0, 0:nw])
                    nc.sync.dma_start(
                        out=out_d[ms * 128:ms * 128 + sp, noff:noff + nw],
                        in_=ob[0:sp, 0:nw])
                    noff += nw

        # ---- interleaved emission: c0+c1 units ride along with projections
        emit_qk(0)
        emit_qk(1)
        emit_v(0)
        for h in (0, 1, 2, 3):
            emit_unit(h, 0)
            emit_unit(h, 1)
        emit_qk(2)
        emit_unit(4, 0)
        emit_unit(4, 1)
        emit_v(1)
        emit_unit(5, 0)
        emit_unit(5, 1)
        emit_qk(3)
        for h in (6, 7):
            emit_unit(h, 0)
            emit_unit(h, 1)
        emit_qk(4)
        for h in (8, 9):
            emit_unit(h, 0)
            emit_unit(h, 1)
        pb.close()  # free xt + weight streaming space

        pdx = ExitStack()
        wopool = pdx.enter_context(tc.tile_pool(name="wo", bufs=1))
        opool = pdx.enter_context(tc.tile_pool(name="ob", bufs=3))
        wo_s = wopool.tile([128, MD, D], dt.float32r, tag="wo")
        nc.gpsimd.dma_start(out=wo_s[:], in_=wo_r[:])

        emit_oproj(range(0, 4), wo_s, opool)     # sq < 512 final after c=0
        for h in range(4):
            emit_unit(h, 2)
        emit_oproj(range(4, 8), wo_s, opool)     # sq < 1024 final after c=1
        for h in range(4, HPG):
            emit_unit(h, 2)
        emit_oproj(range(8, KS), wo_s, opool)
        pdx.close()

    nc.compile()
    return nc


def _get_nc():
    if "nc" not in _CACHE:
        _CACHE["nc"] = build()
    return _CACHE["nc"]


def _prep_in_maps(x, Wq, bq, Wk, Wv, Wo):
    in_maps = []
    for c in range(N_CORES):
        b, g = divmod(c, G)
        gs = slice(g * DG, (g + 1) * DG)
        in_maps.append({
            "xt": np.ascontiguousarray(x[b].T).astype(np.float16),
            "wq": np.ascontiguousarray(Wq[gs, :].T).astype(np.float16),
            "wk": np.ascontiguousarray(Wk[gs, :].T).astype(np.float16),
            "wv": np.ascontiguousarray(Wv[gs, :].T).astype(np.float16),
            "wo": np.ascontiguousarray(Wo[:, gs].T).astype(np.float32),
            "bq": np.ascontiguousarray(
                (0.125 * bq[gs]).astype(np.float32).reshape(MD, 128).T),
        })
    return in_maps


def run(x, Wq, bq, Wk, Wv, bv, Wo, bo, trace=False, **trace_kw):
    x = np.asarray(x, dtype=np.float32)
    Wq = np.asarray(Wq, dtype=np.float32)
    bq = np.asarray(bq, dtype=np.float32)
    Wk = np.asarray(Wk, dtype=np.float32)
    Wv = np.asarray(Wv, dtype=np.float32)
    bv = np.asarray(bv, dtype=np.float32)
    Wo = np.asarray(Wo, dtype=np.float32)
    bo = np.asarray(bo, dtype=np.float32)

    nc = _get_nc()
    in_maps = _prep_in_maps(x, Wq, bq, Wk, Wv, Wo)
    res = None
    for attempt in range(3):
        try:
            res = run_bass_kernel_spmd(nc, in_maps, list(range(N_CORES)),
                                       trace=trace, **trace_kw)
            break
        except Exception:
            # Sporadic NRT_EXEC_UNIT_UNRECOVERABLE on first exec; devices
            # come back after ~75s. Reset the backend and retry.
            if attempt == 2:
                raise
            import time as _time
            import jax as _jax
            _time.sleep(80)
            try:
                _jax.clear_backends()
            except Exception:
                pass
    const = (bv @ Wo.T + bo).astype(np.float32)  # [D]
    out = np.empty((B, S, D), dtype=np.float32)
    for b in range(B):
        out[b] = res.results[2 * b]["out"] + res.results[2 * b + 1]["out"] + const
    return out, res


def kernel(**inputs):
    out, _ = run(**inputs)
    return out

